# revision 15
# baseline (speedup 1.0000x reference)
"""Trainium2 Bass kernel for batched 9x9-token MHSA with decomposed relative
position bias (1x1-conv QKV projection).

Strategy: pure data parallel over batch (B=1024 -> 128 per core x 8 cores).
Per core (all-fp16 GEMM datapath, fp32 PSUM accumulation):
  - QK projection GEMM channel-major (out [o, (b,n)]), fp16, N=324 tiles.
    Relative-position table R = rel_h+rel_w (+ k bias) is folded into K
    during the PSUM->SBUF epilogue, so scores = Q.(K+R) in one matmul.
  - V projection GEMM token-major with cross-batch 128-token stationary
    tiles (full PE width), epilogue writes interleaved [tok, h, d+1] bf16
    with a ones column; per-batch v_aug tiles are then carved out by
    SBUF->SBUF repartition DMAs (contiguous 1040B rows).
  - Scores computed transposed: S^T[m,n] = sum_d k'[d,m] q[d,n] via
    matmul(lhsT=k', rhs=q), fp16 inputs. Softmax runs over partitions (m):
    no max subtraction (logits bounded ~33, exp fits fp32/bf16 range);
    denominator obtained from the ones column of v_aug so the AV matmul
    emits unnormalized output rows 0..63 and the denominator in row 64.
  - exp on ScalarE (fp32 PSUM -> bf16 SBUF), AV matmul in bf16.
  - Final normalize (divide by denominator row) happens on the host during
    unsharding.

Self-contained: hardcodes B=1024, DM=512, H=8, D=64, N=81, 8 cores.
"""

import os
import sys

import numpy as np

for _p in ("/opt/trn_rl_repo", "/root/.axon_site/_ro/trn_rl_repo"):
    if os.path.isdir(_p) and _p not in sys.path:
        sys.path.insert(0, _p)

import concourse.bass as bass
import concourse.tile as tile
from concourse import bacc
from concourse import mybir
from concourse.alu_op_type import AluOpType
from concourse.bass_utils import run_bass_kernel_spmd

F32 = mybir.dt.float32
BF16 = mybir.dt.bfloat16
FP16 = mybir.dt.float16
AF = mybir.ActivationFunctionType

B, DM, H, D, N = 1024, 512, 8, 64, 81
NCORES = 8
B_CORE = B // NCORES   # 128
NB = 16                # batches per chunk
NTOK = NB * N          # 1296 tokens per chunk
NSUB = 4               # QK sub-chunks per chunk
SUBC = NTOK // NSUB    # 324 moving columns per QK matmul
E = D + 1              # 65: v columns + ones (denominator) column


def build_kernel(n_b=B_CORE):
    assert n_b % NB == 0
    nchunks = n_b // NB

    nc = bacc.Bacc()
    # x pre-transposed on host to channel-major [DM, n_b*N] fp16 (contiguous
    # 2592B DMA rows per chunk slice; fp16 keeps every matmul at the
    # 1-cycle/row rate and halves HBM traffic vs fp32).
    xd = nc.dram_tensor("x", [DM, n_b * N], FP16, kind="ExternalInput")
    wtd = nc.dram_tensor("wt", [DM, 3 * DM], FP16, kind="ExternalInput")  # W^T
    bqd = nc.dram_tensor("bq", [DM, 1], F32, kind="ExternalInput")        # q bias
    rpd = nc.dram_tensor("rp", [DM, N], F32, kind="ExternalInput")        # rel_h+rel_w+bk
    bvd = nc.dram_tensor("bv", [1, DM], F32, kind="ExternalInput")        # v bias row
    # out in device-native layout [pair][d+denom][b par hh n]; row D holds the
    # softmax denominator — the final normalize division happens on the host
    # during unsharding. One fully-contiguous store per batch pair.
    outd = nc.dram_tensor(
        "out", [n_b // 2, D + 1, 2 * 2 * 4 * N], BF16, kind="ExternalOutput"
    )

    with tile.TileContext(nc) as tc:
        with (
            tc.tile_pool(name="const", bufs=1) as cpool,
            tc.tile_pool(name="xin", bufs=3) as xpool,
            tc.tile_pool(name="qk", bufs=2) as qkpool,
            tc.tile_pool(name="vtok", bufs=14) as vtpool,
            tc.tile_pool(name="vx", bufs=3) as vxpool,
            tc.tile_pool(name="vaug", bufs=2) as vpool,
            tc.tile_pool(name="emat", bufs=4) as epool,
            tc.tile_pool(name="outs", bufs=3) as opool,
            tc.tile_pool(name="ps_qk", bufs=2, space="PSUM") as ps_qk,
            tc.tile_pool(name="ps_v", bufs=2, space="PSUM") as ps_v,
            tc.tile_pool(name="ps_s", bufs=2, space="PSUM") as ps_s,
            tc.tile_pool(name="ps_av", bufs=2, space="PSUM") as ps_av,
        ):
            # ---- constants (loaded once) ----
            wt = []
            for kc in range(4):
                t = cpool.tile([128, 3 * DM], FP16, tag=f"wt{kc}")
                nc.sync.dma_start(out=t[:], in_=wtd[kc * 128:(kc + 1) * 128, :])
                wt.append(t)
            bq = []
            for mo in range(4):
                t = cpool.tile([128, 1], F32, tag=f"bq{mo}")
                nc.sync.dma_start(out=t[:], in_=bqd[mo * 128:(mo + 1) * 128, :])
                bq.append(t)
            # rp widened to one QK sub-chunk (4 batches) so the k epilogue is
            # a plain 2D tensor_tensor with no broadcast AP
            rpw = []
            for mo in range(4):
                t = cpool.tile([128, SUBC], F32, tag=f"rpw{mo}")
                nc.sync.dma_start(
                    out=t[:].rearrange("p (b n) -> p b n", b=NSUB),
                    in_=rpd[mo * 128:(mo + 1) * 128, :].unsqueeze(1)
                    .to_broadcast([128, NSUB, N]),
                )
                rpw.append(t)
            bvb = cpool.tile([128, DM], F32, tag="bvb")
            nc.sync.dma_start(out=bvb[:], in_=bvd[0:1, :].to_broadcast([128, DM]))

            state = {}  # carries one chunk's tiles to the next iteration
            vt_glob = {}  # global V-tile index -> vtok tile

            def gemm(c):
                b0 = c * NB
                xt = []
                for kc in range(4):
                    t = xpool.tile([128, NTOK], FP16, tag=f"x{kc}")
                    # per-sub-chunk DMAs so the first QK matmuls start after
                    # 1/NSUB of the chunk's x has landed
                    for s in range(NSUB):
                        nc.sync.dma_start(
                            out=t[:, s * SUBC:(s + 1) * SUBC],
                            in_=xd[kc * 128:(kc + 1) * 128,
                                   b0 * N + s * SUBC:b0 * N + (s + 1) * SUBC],
                        )
                    xt.append(t)

                # q,k channel-major GEMM: out[o, (b,n)] for o in 0..1024,
                # NSUB sub-chunks of SUBC moving columns each
                q_sb = [
                    qkpool.tile([128, NTOK], FP16, tag=f"q{mo}", name=f"q{mo}")
                    for mo in range(4)
                ]
                k_sb = [
                    qkpool.tile([128, NTOK], FP16, tag=f"k{mo}", name=f"k{mo}")
                    for mo in range(4)
                ]
                for s in range(NSUB):
                    cs = slice(s * SUBC, (s + 1) * SUBC)
                    for mo in range(8):
                        ps = ps_qk.tile([128, SUBC], F32, tag="psqk")
                        for kc in range(4):
                            nc.tensor.matmul(
                                ps[:],
                                lhsT=wt[kc][:, mo * 128:(mo + 1) * 128],
                                rhs=xt[kc][:, cs],
                                start=(kc == 0),
                                stop=(kc == 3),
                            )
                        if mo < 4:  # q: add bias on ScalarE while copying out
                            nc.scalar.activation(
                                q_sb[mo][:, cs], ps[:], AF.Identity, bias=bq[mo][:]
                            )
                        else:  # k: add (rel bias + k bias)
                            nc.vector.tensor_tensor(
                                k_sb[mo - 4][:, cs], ps[:], rpw[mo - 4][:],
                                AluOpType.add,
                            )

                # v token-major GEMM on GLOBAL 128-token stationary tiles
                # (10368 = 81*128 exactly, so no remainder tiles). x re-read
                # from HBM in [128, 128] slices; tile t emitted in the first
                # chunk that needs it. Epilogue writes interleaved
                # [tok, h, d|1] bf16 with the ones (denominator) column.
                for t in range(n_b * N // 128):
                    if (128 * t) // NTOK != c:
                        continue
                    vx = []
                    for kc in range(4):
                        g = vxpool.tile(
                            [128, 128], FP16, tag=f"vx{kc}", name=f"vx{kc}_{t}"
                        )
                        nc.sync.dma_start(
                            out=g[:],
                            in_=xd[kc * 128:(kc + 1) * 128, 128 * t:128 * (t + 1)],
                        )
                        vx.append(g)
                    ps = ps_v.tile([128, DM], F32, tag="psv")
                    for kc in range(4):
                        nc.tensor.matmul(
                            ps[:],
                            lhsT=vx[kc][:],
                            rhs=wt[kc][:, 2 * DM:3 * DM],
                            start=(kc == 0),
                            stop=(kc == 3),
                        )
                    vt = vtpool.tile([128, H, E], BF16, tag="vtok", name=f"vt{t}")
                    nc.vector.tensor_tensor(
                        vt[:, :, 0:D],
                        ps[:].rearrange("p (h d) -> p h d", h=H),
                        bvb[:].rearrange("p (h d) -> p h d", h=H),
                        AluOpType.add,
                    )
                    nc.vector.memset(vt[:, :, D:E], 1.0)
                    vt_glob[t] = vt

                # repartition: per-batch v_aug tiles [81, H*E] carved out of
                # global vtok tiles by SBUF->SBUF DMA (full 1040B rows)
                v_aug = []
                for j in range(NB):
                    va = vpool.tile([N, H * E], BF16, tag=f"vaug{j}", name=f"va{j}")
                    r0 = (c * NB + j) * N   # global token start
                    for t in range(r0 // 128, (r0 + N - 1) // 128 + 1):
                        lo = max(r0, 128 * t)
                        hi = min(r0 + N, 128 * (t + 1))
                        nc.sync.dma_start(
                            out=va[lo - r0:hi - r0, :],
                            in_=vt_glob[t][lo - 128 * t:hi - 128 * t, :, :]
                            .rearrange("p h e -> p (h e)"),
                        )
                    v_aug.append(va)
                return {"q": q_sb, "k": k_sb, "v": v_aug, "b0": b0}

            def attention(st):
                q_sb, k_sb, v_aug, b0 = st["q"], st["k"], st["v"], st["b0"]
                ot = None
                for j in range(NB):
                    b = b0 + j
                    js = slice(j * N, (j + 1) * N)
                    if j % 2 == 0:  # one output tile per batch pair
                        ot = opool.tile([D + 1, 2 * 2 * 4 * N], BF16, tag="ot")
                    # scores transposed: S^T = k'.T-contracted over d.
                    # Grouped by head parity: tile `par` holds heads 2*hh+par,
                    # so every matmul into one PSUM tile has the same lhsT
                    # base partition; parities interleaved: consecutive
                    # matmuls use disjoint PE row strips (0-63 vs 64-127) and
                    # different PSUM banks, so the PE can overlap them
                    psS = [
                        ps_s.tile([N, 4 * N], F32, tag="pss", name=f"pss{j}_{p}")
                        for p in range(2)
                    ]
                    for hh in range(4):
                        for par in range(2):
                            po = par * 64
                            nc.tensor.matmul(
                                psS[par][:, hh * N:(hh + 1) * N],
                                lhsT=k_sb[hh][po:po + 64, js],
                                rhs=q_sb[hh][po:po + 64, js],
                                start=True,
                                stop=True,
                                tile_position=(po, 0),
                            )
                    emat = []
                    for par in range(2):
                        e = epool.tile([N, 4 * N], BF16, tag="e", name=f"e{par}")
                        nc.scalar.activation(e[:], psS[par][:], AF.Exp)
                        emat.append(e)
                    # AV with ones-row: rows 0..63 unnormalized out, row 64 denom
                    psA = []
                    for par in range(2):
                        ps = ps_av.tile([D + 1, 4 * N], F32, tag="psav")
                        for hh in range(4):
                            h = 2 * hh + par
                            nc.tensor.matmul(
                                ps[:, hh * N:(hh + 1) * N],
                                lhsT=v_aug[j][:, h * E:(h + 1) * E],
                                rhs=emat[par][:, hh * N:(hh + 1) * N],
                                start=True,
                                stop=True,
                            )
                        psA.append(ps)
                    # ot free layout is (b01, par, hh, n); channel h = 2*hh+par
                    # one copy on DVE, one on ACT to balance engine load
                    joff = (j % 2) * 2 * 4 * N
                    nc.vector.tensor_copy(
                        ot[:, joff:joff + 4 * N], psA[0][:]
                    )
                    nc.scalar.activation(
                        ot[:, joff + 4 * N:joff + 2 * 4 * N], psA[1][:], AF.Identity
                    )
                    if j % 2 == 1:
                        nc.sync.dma_start(out=outd[b // 2], in_=ot[:])

            # software pipeline: attention for chunk c-1 is emitted before
            # GEMM for chunk c so PE never stalls on ACT/DVE epilogues
            for c in range(nchunks + 1):
                if c > 0:
                    attention(state)
                if c < nchunks:
                    state = gemm(c)

    if not nc.is_finalized():
        nc.finalize()
    return nc


_CACHE = {}


def _get_nc(n_b):
    if n_b not in _CACHE:
        _CACHE[n_b] = build_kernel(n_b)
    return _CACHE[n_b]


def _prep_inputs(x, qkv_w, qkv_b, rel_h, rel_w):
    # per-core channel-major x: [NCORES][DM, B_CORE*N] in fp16
    x = np.asarray(x, dtype=np.float32).reshape(B, DM, N)
    x = np.ascontiguousarray(
        x.reshape(NCORES, B_CORE, DM, N).transpose(0, 2, 1, 3)
    ).reshape(NCORES, DM, B_CORE * N).astype(np.float16)
    qkv_w = np.asarray(qkv_w, dtype=np.float32)
    qkv_b = np.asarray(qkv_b, dtype=np.float32)
    wt = np.ascontiguousarray(qkv_w.T).astype(np.float16)                # [512, 1536]
    bq = np.ascontiguousarray(qkv_b[0:DM].reshape(DM, 1))
    rel = (np.asarray(rel_h, np.float32) + np.asarray(rel_w, np.float32))
    rp = np.ascontiguousarray(rel.reshape(DM, N) + qkv_b[DM:2 * DM].reshape(DM, 1))
    bv = np.ascontiguousarray(qkv_b[2 * DM:3 * DM].reshape(1, DM))
    return x, wt, bq, rp, bv


def kernel(x, qkv_w, qkv_b, rel_h, rel_w, _trace=False):
    xs, wt, bq, rp, bv = _prep_inputs(x, qkv_w, qkv_b, rel_h, rel_w)
    nc = _get_nc(B_CORE)
    in_maps = [
        {"x": xs[i], "wt": wt, "bq": bq, "rp": rp, "bv": bv}
        for i in range(NCORES)
    ]
    res = run_bass_kernel_spmd(
        nc, in_maps, core_ids=list(range(NCORES)), trace=_trace
    )
    # decode device layout [pair, d|denom, b01, par, hh, n] -> [B, DM, N];
    # row D is the softmax denominator (normalize here during unshard)
    out = np.stack(
        [np.asarray(r["out"]).astype(np.float32) for r in res.results], axis=0
    )
    out = out.reshape(NCORES, B_CORE // 2, D + 1, 2, 2, 4, N)
    out = out[:, :, 0:D] / out[:, :, D:D + 1]
    out = out.transpose(0, 1, 3, 5, 4, 2, 6)  # core, pair, b01, hh, par, d, n
    out = out.reshape(B, DM, N)
    if _trace:
        kernel.last_results = res
    return np.ascontiguousarray(out.reshape(B, DM, 9, 9))


# revision 16
# speedup vs baseline: 1.2194x; 1.2194x over previous
"""Trainium2 Bass kernel for batched 9x9-token MHSA with decomposed relative
position bias (1x1-conv QKV projection).

Strategy: pure data parallel over batch (B=1024 -> 128 per core x 8 cores).
Per core (all-fp16 GEMM datapath, fp32 PSUM accumulation):
  - QK projection GEMM channel-major (out [o, (b,n)]), fp16, N=324 tiles.
    Relative-position table R = rel_h+rel_w (+ k bias) is folded into K
    during the PSUM->SBUF epilogue, so scores = Q.(K+R) in one matmul.
  - V projection GEMM token-major with cross-batch 128-token stationary
    tiles (full PE width), epilogue writes interleaved [tok, h, d+1] bf16
    with a ones column; per-batch v_aug tiles are then carved out by
    SBUF->SBUF repartition DMAs (contiguous 1040B rows).
  - Scores computed transposed: S^T[m,n] = sum_d k'[d,m] q[d,n] via
    matmul(lhsT=k', rhs=q), fp16 inputs. Softmax runs over partitions (m):
    no max subtraction (logits bounded ~33, exp fits fp32/bf16 range);
    denominator obtained from the ones column of v_aug so the AV matmul
    emits unnormalized output rows 0..63 and the denominator in row 64.
  - exp on ScalarE (fp32 PSUM -> bf16 SBUF), AV matmul in bf16.
  - Final normalize (divide by denominator row) happens on the host during
    unsharding.

Self-contained: hardcodes B=1024, DM=512, H=8, D=64, N=81, 8 cores.
"""

import os
import sys

import numpy as np

for _p in ("/opt/trn_rl_repo", "/root/.axon_site/_ro/trn_rl_repo"):
    if os.path.isdir(_p) and _p not in sys.path:
        sys.path.insert(0, _p)

import concourse.bass as bass
import concourse.tile as tile
from concourse import bacc
from concourse import mybir
from concourse.alu_op_type import AluOpType
from concourse.bass_utils import run_bass_kernel_spmd

F32 = mybir.dt.float32
BF16 = mybir.dt.bfloat16
FP16 = mybir.dt.float16
AF = mybir.ActivationFunctionType

B, DM, H, D, N = 1024, 512, 8, 64, 81
NCORES = 8
B_CORE = B // NCORES   # 128
NB = 16                # batches per chunk
NTOK = NB * N          # 1296 tokens per chunk
NSUB = 4               # QK sub-chunks per chunk
SUBC = NTOK // NSUB    # 324 moving columns per QK matmul
E = D + 1              # 65: v columns + ones (denominator) column


def build_kernel(n_b=B_CORE):
    assert n_b % NB == 0
    nchunks = n_b // NB

    nc = bacc.Bacc()
    # x pre-transposed on host to channel-major [DM, n_b*N] fp16 (contiguous
    # 2592B DMA rows per chunk slice; fp16 keeps every matmul at the
    # 1-cycle/row rate and halves HBM traffic vs fp32).
    xd = nc.dram_tensor("x", [DM, n_b * N], FP16, kind="ExternalInput")
    wtd = nc.dram_tensor("wt", [DM, 3 * DM], FP16, kind="ExternalInput")  # W^T
    bqd = nc.dram_tensor("bq", [DM, 1], F32, kind="ExternalInput")        # q bias
    rpd = nc.dram_tensor("rp", [DM, N], F32, kind="ExternalInput")        # rel_h+rel_w+bk
    bvd = nc.dram_tensor("bv", [1, DM], F32, kind="ExternalInput")        # v bias row
    # out in device-native layout [pair][d+denom][b par hh n]; row D holds the
    # softmax denominator — the final normalize division happens on the host
    # during unsharding. One fully-contiguous store per batch pair.
    outd = nc.dram_tensor(
        "out", [n_b // 2, D + 1, 2 * 2 * 4 * N], BF16, kind="ExternalOutput"
    )

    with tile.TileContext(nc) as tc:
        with (
            tc.tile_pool(name="const", bufs=1) as cpool,
            tc.tile_pool(name="xin", bufs=3) as xpool,
            tc.tile_pool(name="qk", bufs=2) as qkpool,
            tc.tile_pool(name="vtok", bufs=14) as vtpool,
            tc.tile_pool(name="vx", bufs=3) as vxpool,
            tc.tile_pool(name="vaug", bufs=2) as vpool,
            tc.tile_pool(name="emat", bufs=4) as epool,
            tc.tile_pool(name="outs", bufs=3) as opool,
            tc.tile_pool(name="ps_qk", bufs=2, space="PSUM") as ps_qk,
            tc.tile_pool(name="ps_v", bufs=2, space="PSUM") as ps_v,
            tc.tile_pool(name="ps_s", bufs=2, space="PSUM") as ps_s,
            tc.tile_pool(name="ps_av", bufs=2, space="PSUM") as ps_av,
        ):
            # ---- constants (loaded once) ----
            wt = []
            for kc in range(4):
                t = cpool.tile([128, 3 * DM], FP16, tag=f"wt{kc}")
                nc.sync.dma_start(out=t[:], in_=wtd[kc * 128:(kc + 1) * 128, :])
                wt.append(t)
            bq = []
            for mo in range(4):
                t = cpool.tile([128, 1], F32, tag=f"bq{mo}")
                nc.sync.dma_start(out=t[:], in_=bqd[mo * 128:(mo + 1) * 128, :])
                bq.append(t)
            # rp widened to one QK sub-chunk (4 batches) so the k epilogue is
            # a plain 2D tensor_tensor with no broadcast AP
            rpw = []
            for mo in range(4):
                t = cpool.tile([128, SUBC], F32, tag=f"rpw{mo}")
                nc.sync.dma_start(
                    out=t[:].rearrange("p (b n) -> p b n", b=NSUB),
                    in_=rpd[mo * 128:(mo + 1) * 128, :].unsqueeze(1)
                    .to_broadcast([128, NSUB, N]),
                )
                rpw.append(t)
            bvb = cpool.tile([128, DM], F32, tag="bvb")
            nc.sync.dma_start(out=bvb[:], in_=bvd[0:1, :].to_broadcast([128, DM]))

            state = {}  # carries one chunk's tiles to the next iteration
            vt_glob = {}  # global V-tile index -> vtok tile

            def gemm(c):
                b0 = c * NB
                xt = []
                for kc in range(4):
                    t = xpool.tile([128, NTOK], FP16, tag=f"x{kc}")
                    # per-sub-chunk DMAs so the first QK matmuls start after
                    # 1/NSUB of the chunk's x has landed
                    for s in range(NSUB):
                        nc.sync.dma_start(
                            out=t[:, s * SUBC:(s + 1) * SUBC],
                            in_=xd[kc * 128:(kc + 1) * 128,
                                   b0 * N + s * SUBC:b0 * N + (s + 1) * SUBC],
                        )
                    xt.append(t)

                # q,k channel-major GEMM: out[o, (b,n)] for o in 0..1024,
                # NSUB sub-chunks of SUBC moving columns each
                q_sb = [
                    qkpool.tile([128, NTOK], FP16, tag=f"q{mo}", name=f"q{mo}")
                    for mo in range(4)
                ]
                k_sb = [
                    qkpool.tile([128, NTOK], FP16, tag=f"k{mo}", name=f"k{mo}")
                    for mo in range(4)
                ]
                for s in range(NSUB):
                    cs = slice(s * SUBC, (s + 1) * SUBC)
                    for mo in range(8):
                        ps = ps_qk.tile([128, SUBC], F32, tag="psqk")
                        for kc in range(4):
                            nc.tensor.matmul(
                                ps[:],
                                lhsT=wt[kc][:, mo * 128:(mo + 1) * 128],
                                rhs=xt[kc][:, cs],
                                start=(kc == 0),
                                stop=(kc == 3),
                            )
                        if mo < 4:  # q: add bias on ScalarE while copying out
                            nc.scalar.activation(
                                q_sb[mo][:, cs], ps[:], AF.Identity, bias=bq[mo][:]
                            )
                        else:  # k: add (rel bias + k bias)
                            nc.vector.tensor_tensor(
                                k_sb[mo - 4][:, cs], ps[:], rpw[mo - 4][:],
                                AluOpType.add,
                            )

                # v token-major GEMM with cross-batch 128-token stationary
                # tiles; epilogue writes interleaved [tok, h, d|1] bf16
                vtoks = []   # (tok0, tile)
                for t0 in range(0, NTOK, 128):
                    t1 = min(t0 + 128, NTOK)
                    w = t1 - t0
                    ps = ps_v.tile([128, DM], F32, tag="psv")
                    for kc in range(4):
                        nc.tensor.matmul(
                            ps[0:w, :],
                            lhsT=xt[kc][:, t0:t1],
                            rhs=wt[kc][:, 2 * DM:3 * DM],
                            start=(kc == 0),
                            stop=(kc == 3),
                        )
                    vt = vtpool.tile([128, H, E], BF16, tag="vtok")
                    nc.vector.tensor_tensor(
                        vt[0:w, :, 0:D],
                        ps[0:w, :].rearrange("p (h d) -> p h d", h=H),
                        bvb[0:w, :].rearrange("p (h d) -> p h d", h=H),
                        AluOpType.add,
                    )
                    nc.vector.memset(vt[0:w, :, D:E], 1.0)
                    vtoks.append((t0, vt))

                # repartition: per-batch v_aug tiles [81, H*E] carved out of
                # vtok tiles by SBUF->SBUF DMA (full 1040B rows)
                v_aug = []
                for j in range(NB):
                    va = vpool.tile([N, H * E], BF16, tag=f"vaug{j}", name=f"va{j}")
                    r0 = j * N
                    for t0, vt in vtoks:
                        lo = max(r0, t0)
                        hi = min(r0 + N, t0 + 128)
                        if lo < hi:
                            nc.sync.dma_start(
                                out=va[lo - r0:hi - r0, :],
                                in_=vt[lo - t0:hi - t0, :, :]
                                .rearrange("p h e -> p (h e)"),
                            )
                    v_aug.append(va)
                return {"q": q_sb, "k": k_sb, "v": v_aug, "b0": b0}

            def attention(st):
                q_sb, k_sb, v_aug, b0 = st["q"], st["k"], st["v"], st["b0"]
                ot = None
                for j in range(NB):
                    b = b0 + j
                    js = slice(j * N, (j + 1) * N)
                    if j % 2 == 0:  # one output tile per batch pair
                        ot = opool.tile([D + 1, 2 * 2 * 4 * N], BF16, tag="ot")
                    # scores transposed: S^T = k'.T-contracted over d.
                    # Grouped by head parity: tile `par` holds heads 2*hh+par,
                    # so every matmul into one PSUM tile has the same lhsT
                    # base partition; parities interleaved: consecutive
                    # matmuls use disjoint PE row strips (0-63 vs 64-127) and
                    # different PSUM banks, so the PE can overlap them
                    psS = [
                        ps_s.tile([N, 4 * N], F32, tag="pss", name=f"pss{j}_{p}")
                        for p in range(2)
                    ]
                    for hh in range(4):
                        for par in range(2):
                            po = par * 64
                            nc.tensor.matmul(
                                psS[par][:, hh * N:(hh + 1) * N],
                                lhsT=k_sb[hh][po:po + 64, js],
                                rhs=q_sb[hh][po:po + 64, js],
                                start=True,
                                stop=True,
                                tile_position=(po, 0),
                            )
                    emat = []
                    for par in range(2):
                        e = epool.tile([N, 4 * N], BF16, tag="e", name=f"e{par}")
                        nc.scalar.activation(e[:], psS[par][:], AF.Exp)
                        emat.append(e)
                    # AV with ones-row: rows 0..63 unnormalized out, row 64 denom
                    psA = []
                    for par in range(2):
                        ps = ps_av.tile([D + 1, 4 * N], F32, tag="psav")
                        for hh in range(4):
                            h = 2 * hh + par
                            nc.tensor.matmul(
                                ps[:, hh * N:(hh + 1) * N],
                                lhsT=v_aug[j][:, h * E:(h + 1) * E],
                                rhs=emat[par][:, hh * N:(hh + 1) * N],
                                start=True,
                                stop=True,
                            )
                        psA.append(ps)
                    # ot free layout is (b01, par, hh, n); channel h = 2*hh+par
                    # one copy on DVE, one on ACT to balance engine load
                    joff = (j % 2) * 2 * 4 * N
                    nc.vector.tensor_copy(
                        ot[:, joff:joff + 4 * N], psA[0][:]
                    )
                    nc.scalar.activation(
                        ot[:, joff + 4 * N:joff + 2 * 4 * N], psA[1][:], AF.Identity
                    )
                    if j % 2 == 1:
                        nc.sync.dma_start(out=outd[b // 2], in_=ot[:])

            # software pipeline: attention for chunk c-1 is emitted before
            # GEMM for chunk c so PE never stalls on ACT/DVE epilogues
            for c in range(nchunks + 1):
                if c > 0:
                    attention(state)
                if c < nchunks:
                    state = gemm(c)

    if not nc.is_finalized():
        nc.finalize()
    return nc


_CACHE = {}


def _get_nc(n_b):
    if n_b not in _CACHE:
        _CACHE[n_b] = build_kernel(n_b)
    return _CACHE[n_b]


def _prep_inputs(x, qkv_w, qkv_b, rel_h, rel_w):
    # per-core channel-major x: [NCORES][DM, B_CORE*N] in fp16
    x = np.asarray(x, dtype=np.float32).reshape(B, DM, N)
    x = np.ascontiguousarray(
        x.reshape(NCORES, B_CORE, DM, N).transpose(0, 2, 1, 3)
    ).reshape(NCORES, DM, B_CORE * N).astype(np.float16)
    qkv_w = np.asarray(qkv_w, dtype=np.float32)
    qkv_b = np.asarray(qkv_b, dtype=np.float32)
    wt = np.ascontiguousarray(qkv_w.T).astype(np.float16)                # [512, 1536]
    bq = np.ascontiguousarray(qkv_b[0:DM].reshape(DM, 1))
    rel = (np.asarray(rel_h, np.float32) + np.asarray(rel_w, np.float32))
    rp = np.ascontiguousarray(rel.reshape(DM, N) + qkv_b[DM:2 * DM].reshape(DM, 1))
    bv = np.ascontiguousarray(qkv_b[2 * DM:3 * DM].reshape(1, DM))
    return x, wt, bq, rp, bv


def kernel(x, qkv_w, qkv_b, rel_h, rel_w, _trace=False):
    xs, wt, bq, rp, bv = _prep_inputs(x, qkv_w, qkv_b, rel_h, rel_w)
    nc = _get_nc(B_CORE)
    in_maps = [
        {"x": xs[i], "wt": wt, "bq": bq, "rp": rp, "bv": bv}
        for i in range(NCORES)
    ]
    res = run_bass_kernel_spmd(
        nc, in_maps, core_ids=list(range(NCORES)), trace=_trace
    )
    # decode device layout [pair, d|denom, b01, par, hh, n] -> [B, DM, N];
    # row D is the softmax denominator (normalize here during unshard)
    out = np.stack(
        [np.asarray(r["out"]).astype(np.float32) for r in res.results], axis=0
    )
    out = out.reshape(NCORES, B_CORE // 2, D + 1, 2, 2, 4, N)
    out = out[:, :, 0:D] / out[:, :, D:D + 1]
    out = out.transpose(0, 1, 3, 5, 4, 2, 6)  # core, pair, b01, hh, par, d, n
    out = out.reshape(B, DM, N)
    if _trace:
        kernel.last_results = res
    return np.ascontiguousarray(out.reshape(B, DM, 9, 9))


# revision 18
# speedup vs baseline: 1.2839x; 1.0529x over previous
"""Trainium2 Bass kernel for batched 9x9-token MHSA with decomposed relative
position bias (1x1-conv QKV projection).

Strategy: pure data parallel over batch (B=1024 -> 128 per core x 8 cores).
Per core (all-fp16 GEMM datapath, fp32 PSUM accumulation):
  - QK projection GEMM channel-major (out [o, (b,n)]), fp16, N=324 tiles.
    Relative-position table R = rel_h+rel_w (+ k bias) is folded into K
    during the PSUM->SBUF epilogue, so scores = Q.(K+R) in one matmul.
  - V projection GEMM token-major with cross-batch 128-token stationary
    tiles (full PE width), epilogue writes interleaved [tok, h, d+1] bf16
    with a ones column; per-batch v_aug tiles are then carved out by
    SBUF->SBUF repartition DMAs (contiguous 1040B rows).
  - Scores computed transposed: S^T[m,n] = sum_d k'[d,m] q[d,n] via
    matmul(lhsT=k', rhs=q), fp16 inputs. Softmax runs over partitions (m):
    no max subtraction (logits bounded ~33, exp fits fp32/bf16 range);
    denominator obtained from the ones column of v_aug so the AV matmul
    emits unnormalized output rows 0..63 and the denominator in row 64.
  - exp on ScalarE (fp32 PSUM -> bf16 SBUF), AV matmul in bf16.
  - Final normalize (divide by denominator row) happens on the host during
    unsharding.

Self-contained: hardcodes B=1024, DM=512, H=8, D=64, N=81, 8 cores.
"""

import os
import sys

import numpy as np

for _p in ("/opt/trn_rl_repo", "/root/.axon_site/_ro/trn_rl_repo"):
    if os.path.isdir(_p) and _p not in sys.path:
        sys.path.insert(0, _p)

import concourse.bass as bass
import concourse.tile as tile
from concourse import bacc
from concourse import mybir
from concourse.alu_op_type import AluOpType
from concourse.bass_utils import run_bass_kernel_spmd

F32 = mybir.dt.float32
BF16 = mybir.dt.bfloat16
FP16 = mybir.dt.float16
AF = mybir.ActivationFunctionType

B, DM, H, D, N = 1024, 512, 8, 64, 81
NCORES = 8
B_CORE = B // NCORES   # 128
NB = 16                # batches per chunk
NTOK = NB * N          # 1296 tokens per chunk
NSUB = 4               # QK sub-chunks per chunk
SUBC = NTOK // NSUB    # 324 moving columns per QK matmul
E = D + 1              # 65: v columns + ones (denominator) column


def build_kernel(n_b=B_CORE):
    assert n_b % NB == 0
    nchunks = n_b // NB

    nc = bacc.Bacc()
    # x pre-transposed on host to channel-major [DM, n_b*N] fp16 (contiguous
    # 2592B DMA rows per chunk slice; fp16 keeps every matmul at the
    # 1-cycle/row rate and halves HBM traffic vs fp32).
    xd = nc.dram_tensor("x", [DM, n_b * N], FP16, kind="ExternalInput")
    wtd = nc.dram_tensor("wt", [DM, 3 * DM], FP16, kind="ExternalInput")  # W^T
    bqd = nc.dram_tensor("bq", [DM, 1], F32, kind="ExternalInput")        # q bias
    rpd = nc.dram_tensor("rp", [DM, N], F32, kind="ExternalInput")        # rel_h+rel_w+bk
    bvd = nc.dram_tensor("bv", [1, DM], F32, kind="ExternalInput")        # v bias row
    # out in device-native layout [pair][d+denom][b par hh n]; row D holds the
    # softmax denominator — the final normalize division happens on the host
    # during unsharding. One fully-contiguous store per batch pair.
    outd = nc.dram_tensor(
        "out", [n_b // 2, D + 1, 2 * 2 * 4 * N], BF16, kind="ExternalOutput"
    )

    with tile.TileContext(nc) as tc:
        with (
            tc.tile_pool(name="const", bufs=1) as cpool,
            tc.tile_pool(name="xin", bufs=3) as xpool,
            tc.tile_pool(name="qk", bufs=2) as qkpool,
            tc.tile_pool(name="vtok", bufs=4) as vtpool,
            tc.tile_pool(name="vaug", bufs=2) as vpool,
            tc.tile_pool(name="emat", bufs=4) as epool,
            tc.tile_pool(name="outs", bufs=3) as opool,
            tc.tile_pool(name="ps_qk", bufs=2, space="PSUM") as ps_qk,
            tc.tile_pool(name="ps_v", bufs=2, space="PSUM") as ps_v,
            tc.tile_pool(name="ps_s", bufs=2, space="PSUM") as ps_s,
            tc.tile_pool(name="ps_av", bufs=2, space="PSUM") as ps_av,
        ):
            # ---- constants (loaded once) ----
            wt = []
            for kc in range(4):
                t = cpool.tile([128, 3 * DM], FP16, tag=f"wt{kc}")
                nc.sync.dma_start(out=t[:], in_=wtd[kc * 128:(kc + 1) * 128, :])
                wt.append(t)
            bq = []
            for mo in range(4):
                t = cpool.tile([128, 1], F32, tag=f"bq{mo}")
                nc.sync.dma_start(out=t[:], in_=bqd[mo * 128:(mo + 1) * 128, :])
                bq.append(t)
            # rp widened to one QK sub-chunk (4 batches) so the k epilogue is
            # a plain 2D tensor_tensor with no broadcast AP
            rpw = []
            for mo in range(4):
                t = cpool.tile([128, SUBC], F32, tag=f"rpw{mo}")
                nc.sync.dma_start(
                    out=t[:].rearrange("p (b n) -> p b n", b=NSUB),
                    in_=rpd[mo * 128:(mo + 1) * 128, :].unsqueeze(1)
                    .to_broadcast([128, NSUB, N]),
                )
                rpw.append(t)
            bvb = cpool.tile([128, DM], F32, tag="bvb")
            nc.sync.dma_start(out=bvb[:], in_=bvd[0:1, :].to_broadcast([128, DM]))

            state = {}  # carries one chunk's tiles to the next iteration

            def gemm(c):
                b0 = c * NB
                xt = []
                for kc in range(4):
                    t = xpool.tile([128, NTOK], FP16, tag=f"x{kc}")
                    nc.sync.dma_start(
                        out=t[:],
                        in_=xd[kc * 128:(kc + 1) * 128, b0 * N:(b0 + NB) * N],
                    )
                    xt.append(t)

                # q,k channel-major GEMM: out[o, (b,n)] for o in 0..1024,
                # NSUB sub-chunks of SUBC moving columns each
                q_sb = [
                    qkpool.tile([128, NTOK], FP16, tag=f"q{mo}", name=f"q{mo}")
                    for mo in range(4)
                ]
                k_sb = [
                    qkpool.tile([128, NTOK], FP16, tag=f"k{mo}", name=f"k{mo}")
                    for mo in range(4)
                ]
                for s in range(NSUB):
                    cs = slice(s * SUBC, (s + 1) * SUBC)
                    for mo in range(8):
                        ps = ps_qk.tile([128, SUBC], F32, tag="psqk")
                        for kc in range(4):
                            nc.tensor.matmul(
                                ps[:],
                                lhsT=wt[kc][:, mo * 128:(mo + 1) * 128],
                                rhs=xt[kc][:, cs],
                                start=(kc == 0),
                                stop=(kc == 3),
                            )
                        if mo < 4:  # q: add bias on ScalarE while copying out
                            nc.scalar.activation(
                                q_sb[mo][:, cs], ps[:], AF.Identity, bias=bq[mo][:]
                            )
                        else:  # k: add (rel bias + k bias)
                            nc.vector.tensor_tensor(
                                k_sb[mo - 4][:, cs], ps[:], rpw[mo - 4][:],
                                AluOpType.add,
                            )

                # v token-major GEMM with cross-batch 128-token stationary
                # tiles; epilogue writes interleaved [tok, h, d|1] bf16
                vtoks = []   # (tok0, tile)
                for t0 in range(0, NTOK, 128):
                    t1 = min(t0 + 128, NTOK)
                    w = t1 - t0
                    ps = ps_v.tile([128, DM], F32, tag="psv")
                    for kc in range(4):
                        nc.tensor.matmul(
                            ps[0:w, :],
                            lhsT=xt[kc][:, t0:t1],
                            rhs=wt[kc][:, 2 * DM:3 * DM],
                            start=(kc == 0),
                            stop=(kc == 3),
                        )
                    vt = vtpool.tile([128, H, E], BF16, tag="vtok")
                    nc.vector.tensor_tensor(
                        vt[0:w, :, 0:D],
                        ps[0:w, :].rearrange("p (h d) -> p h d", h=H),
                        bvb[0:w, :].rearrange("p (h d) -> p h d", h=H),
                        AluOpType.add,
                    )
                    nc.vector.memset(vt[0:w, :, D:E], 1.0)
                    vtoks.append((t0, vt))

                # repartition: per-batch v_aug tiles [81, H*E] carved out of
                # vtok tiles by SBUF->SBUF DMA (full 1040B rows)
                v_aug = []
                for j in range(NB):
                    va = vpool.tile([N, H * E], BF16, tag=f"vaug{j}", name=f"va{j}")
                    r0 = j * N
                    for t0, vt in vtoks:
                        lo = max(r0, t0)
                        hi = min(r0 + N, t0 + 128)
                        if lo < hi:
                            nc.sync.dma_start(
                                out=va[lo - r0:hi - r0, :],
                                in_=vt[lo - t0:hi - t0, :, :]
                                .rearrange("p h e -> p (h e)"),
                            )
                    v_aug.append(va)
                return {"q": q_sb, "k": k_sb, "v": v_aug, "b0": b0}

            def attention(st):
                q_sb, k_sb, v_aug, b0 = st["q"], st["k"], st["v"], st["b0"]
                ot = None
                for j in range(NB):
                    b = b0 + j
                    js = slice(j * N, (j + 1) * N)
                    if j % 2 == 0:  # one output tile per batch pair
                        ot = opool.tile([D + 1, 2 * 2 * 4 * N], BF16, tag="ot")
                    # scores transposed: S^T = k'.T-contracted over d.
                    # Grouped by head parity: tile `par` holds heads 2*hh+par,
                    # so every matmul into one PSUM tile has the same lhsT
                    # base partition; parities interleaved: consecutive
                    # matmuls use disjoint PE row strips (0-63 vs 64-127) and
                    # different PSUM banks, so the PE can overlap them
                    psS = [
                        ps_s.tile([N, 4 * N], F32, tag="pss", name=f"pss{j}_{p}")
                        for p in range(2)
                    ]
                    for hh in range(4):
                        for par in range(2):
                            po = par * 64
                            nc.tensor.matmul(
                                psS[par][:, hh * N:(hh + 1) * N],
                                lhsT=k_sb[hh][po:po + 64, js],
                                rhs=q_sb[hh][po:po + 64, js],
                                start=True,
                                stop=True,
                                tile_position=(po, 0),
                            )
                    emat = []
                    for par in range(2):
                        e = epool.tile([N, 4 * N], BF16, tag="e", name=f"e{par}")
                        nc.scalar.activation(e[:], psS[par][:], AF.Exp)
                        emat.append(e)
                    # AV with ones-row: rows 0..63 unnormalized out, row 64 denom
                    psA = []
                    for par in range(2):
                        ps = ps_av.tile([D + 1, 4 * N], F32, tag="psav")
                        for hh in range(4):
                            h = 2 * hh + par
                            nc.tensor.matmul(
                                ps[:, hh * N:(hh + 1) * N],
                                lhsT=v_aug[j][:, h * E:(h + 1) * E],
                                rhs=emat[par][:, hh * N:(hh + 1) * N],
                                start=True,
                                stop=True,
                            )
                        psA.append(ps)
                    # ot free layout is (b01, par, hh, n); channel h = 2*hh+par
                    # one copy on DVE, one on ACT to balance engine load
                    joff = (j % 2) * 2 * 4 * N
                    nc.vector.tensor_copy(
                        ot[:, joff:joff + 4 * N], psA[0][:]
                    )
                    nc.scalar.activation(
                        ot[:, joff + 4 * N:joff + 2 * 4 * N], psA[1][:], AF.Identity
                    )
                    if j % 2 == 1:
                        nc.sync.dma_start(out=outd[b // 2], in_=ot[:])

            # software pipeline: attention for chunk c-1 is emitted before
            # GEMM for chunk c so PE never stalls on ACT/DVE epilogues
            for c in range(nchunks + 1):
                if c > 0:
                    attention(state)
                if c < nchunks:
                    state = gemm(c)

    if not nc.is_finalized():
        nc.finalize()
    return nc


_CACHE = {}


def _get_nc(n_b):
    if n_b not in _CACHE:
        _CACHE[n_b] = build_kernel(n_b)
    return _CACHE[n_b]


def _prep_inputs(x, qkv_w, qkv_b, rel_h, rel_w):
    # per-core channel-major x: [NCORES][DM, B_CORE*N] in fp16
    x = np.asarray(x, dtype=np.float32).reshape(B, DM, N)
    x = np.ascontiguousarray(
        x.reshape(NCORES, B_CORE, DM, N).transpose(0, 2, 1, 3)
    ).reshape(NCORES, DM, B_CORE * N).astype(np.float16)
    qkv_w = np.asarray(qkv_w, dtype=np.float32)
    qkv_b = np.asarray(qkv_b, dtype=np.float32)
    wt = np.ascontiguousarray(qkv_w.T).astype(np.float16)                # [512, 1536]
    bq = np.ascontiguousarray(qkv_b[0:DM].reshape(DM, 1))
    rel = (np.asarray(rel_h, np.float32) + np.asarray(rel_w, np.float32))
    rp = np.ascontiguousarray(rel.reshape(DM, N) + qkv_b[DM:2 * DM].reshape(DM, 1))
    bv = np.ascontiguousarray(qkv_b[2 * DM:3 * DM].reshape(1, DM))
    return x, wt, bq, rp, bv


def kernel(x, qkv_w, qkv_b, rel_h, rel_w, _trace=False):
    xs, wt, bq, rp, bv = _prep_inputs(x, qkv_w, qkv_b, rel_h, rel_w)
    nc = _get_nc(B_CORE)
    in_maps = [
        {"x": xs[i], "wt": wt, "bq": bq, "rp": rp, "bv": bv}
        for i in range(NCORES)
    ]
    res = run_bass_kernel_spmd(
        nc, in_maps, core_ids=list(range(NCORES)), trace=_trace
    )
    # decode device layout [pair, d|denom, b01, par, hh, n] -> [B, DM, N];
    # row D is the softmax denominator (normalize here during unshard)
    out = np.stack(
        [np.asarray(r["out"]).astype(np.float32) for r in res.results], axis=0
    )
    out = out.reshape(NCORES, B_CORE // 2, D + 1, 2, 2, 4, N)
    out = out[:, :, 0:D] / out[:, :, D:D + 1]
    out = out.transpose(0, 1, 3, 5, 4, 2, 6)  # core, pair, b01, hh, par, d, n
    out = out.reshape(B, DM, N)
    if _trace:
        kernel.last_results = res
    return np.ascontiguousarray(out.reshape(B, DM, 9, 9))


# revision 19
# speedup vs baseline: 1.2869x; 1.0023x over previous
"""Trainium2 Bass kernel for batched 9x9-token MHSA with decomposed relative
position bias (1x1-conv QKV projection).

Strategy: pure data parallel over batch (B=1024 -> 128 per core x 8 cores).
Per core (all-fp16 GEMM datapath, fp32 PSUM accumulation):
  - QK projection GEMM channel-major (out [o, (b,n)]), fp16, N=324 tiles.
    Relative-position table R = rel_h+rel_w (+ k bias) is folded into K
    during the PSUM->SBUF epilogue, so scores = Q.(K+R) in one matmul.
  - V projection GEMM token-major with cross-batch 128-token stationary
    tiles (full PE width), epilogue writes interleaved [tok, h, d+1] bf16
    with a ones column; per-batch v_aug tiles are then carved out by
    SBUF->SBUF repartition DMAs (contiguous 1040B rows).
  - Scores computed transposed: S^T[m,n] = sum_d k'[d,m] q[d,n] via
    matmul(lhsT=k', rhs=q), fp16 inputs. Softmax runs over partitions (m):
    no max subtraction (logits bounded ~33, exp fits fp32/bf16 range);
    denominator obtained from the ones column of v_aug so the AV matmul
    emits unnormalized output rows 0..63 and the denominator in row 64.
  - exp on ScalarE (fp32 PSUM -> bf16 SBUF), AV matmul in bf16.
  - Final normalize (divide by denominator row) happens on the host during
    unsharding.

Self-contained: hardcodes B=1024, DM=512, H=8, D=64, N=81, 8 cores.
"""

import os
import sys

import numpy as np

for _p in ("/opt/trn_rl_repo", "/root/.axon_site/_ro/trn_rl_repo"):
    if os.path.isdir(_p) and _p not in sys.path:
        sys.path.insert(0, _p)

import concourse.bass as bass
import concourse.tile as tile
from concourse import bacc
from concourse import mybir
from concourse.alu_op_type import AluOpType
from concourse.bass_utils import run_bass_kernel_spmd

F32 = mybir.dt.float32
BF16 = mybir.dt.bfloat16
FP16 = mybir.dt.float16
AF = mybir.ActivationFunctionType

B, DM, H, D, N = 1024, 512, 8, 64, 81
NCORES = 8
B_CORE = B // NCORES   # 128
NB = 16                # batches per chunk
NTOK = NB * N          # 1296 tokens per chunk
NSUB = 4               # QK sub-chunks per chunk
SUBC = NTOK // NSUB    # 324 moving columns per QK matmul
E = D + 1              # 65: v columns + ones (denominator) column


def build_kernel(n_b=B_CORE):
    assert n_b % NB == 0
    nchunks = n_b // NB

    nc = bacc.Bacc()
    # x pre-transposed on host to channel-major [DM, n_b*N] fp16 (contiguous
    # 2592B DMA rows per chunk slice; fp16 keeps every matmul at the
    # 1-cycle/row rate and halves HBM traffic vs fp32).
    xd = nc.dram_tensor("x", [DM, n_b * N], FP16, kind="ExternalInput")
    wtd = nc.dram_tensor("wt", [DM, 3 * DM], FP16, kind="ExternalInput")  # W^T
    bqd = nc.dram_tensor("bq", [DM, 1], F32, kind="ExternalInput")        # q bias
    rpd = nc.dram_tensor("rp", [DM, N], F32, kind="ExternalInput")        # rel_h+rel_w+bk
    bvd = nc.dram_tensor("bv", [1, DM], F32, kind="ExternalInput")        # v bias row
    # out in device-native layout [pair][d+denom][b par hh n]; row D holds the
    # softmax denominator — the final normalize division happens on the host
    # during unsharding. One fully-contiguous store per batch pair.
    outd = nc.dram_tensor(
        "out", [n_b // 2, D + 1, 2 * 2 * 4 * N], BF16, kind="ExternalOutput"
    )

    with tile.TileContext(nc) as tc:
        with (
            tc.tile_pool(name="const", bufs=1) as cpool,
            tc.tile_pool(name="xin", bufs=3) as xpool,
            tc.tile_pool(name="qk", bufs=2) as qkpool,
            tc.tile_pool(name="vtok", bufs=4) as vtpool,
            tc.tile_pool(name="vaug", bufs=2) as vpool,
            tc.tile_pool(name="emat", bufs=4) as epool,
            tc.tile_pool(name="outs", bufs=3) as opool,
            tc.tile_pool(name="ps_qk", bufs=2, space="PSUM") as ps_qk,
            tc.tile_pool(name="ps_v", bufs=2, space="PSUM") as ps_v,
            tc.tile_pool(name="ps_s", bufs=2, space="PSUM") as ps_s,
            tc.tile_pool(name="ps_av", bufs=2, space="PSUM") as ps_av,
        ):
            # ---- constants (loaded once) ----
            wt = []
            for kc in range(4):
                t = cpool.tile([128, 3 * DM], FP16, tag=f"wt{kc}")
                nc.sync.dma_start(out=t[:], in_=wtd[kc * 128:(kc + 1) * 128, :])
                wt.append(t)
            bq = []
            for mo in range(4):
                t = cpool.tile([128, 1], F32, tag=f"bq{mo}")
                nc.sync.dma_start(out=t[:], in_=bqd[mo * 128:(mo + 1) * 128, :])
                bq.append(t)
            # rp widened to one QK sub-chunk (4 batches) so the k epilogue is
            # a plain 2D tensor_tensor with no broadcast AP
            rpw = []
            for mo in range(4):
                t = cpool.tile([128, SUBC], F32, tag=f"rpw{mo}")
                nc.sync.dma_start(
                    out=t[:].rearrange("p (b n) -> p b n", b=NSUB),
                    in_=rpd[mo * 128:(mo + 1) * 128, :].unsqueeze(1)
                    .to_broadcast([128, NSUB, N]),
                )
                rpw.append(t)
            bvb = cpool.tile([128, DM], F32, tag="bvb")
            nc.sync.dma_start(out=bvb[:], in_=bvd[0:1, :].to_broadcast([128, DM]))

            state = {}  # carries one chunk's tiles to the next iteration

            def gemm(c):
                b0 = c * NB
                xt = []
                for kc in range(4):
                    t = xpool.tile([128, NTOK], FP16, tag=f"x{kc}")
                    nc.sync.dma_start(
                        out=t[:],
                        in_=xd[kc * 128:(kc + 1) * 128, b0 * N:(b0 + NB) * N],
                    )
                    xt.append(t)

                # q,k channel-major GEMM: out[o, (b,n)] for o in 0..1024,
                # NSUB sub-chunks of SUBC moving columns each
                q_sb = [
                    qkpool.tile([128, NTOK], FP16, tag=f"q{mo}", name=f"q{mo}")
                    for mo in range(4)
                ]
                k_sb = [
                    qkpool.tile([128, NTOK], FP16, tag=f"k{mo}", name=f"k{mo}")
                    for mo in range(4)
                ]
                for s in range(NSUB):
                    cs = slice(s * SUBC, (s + 1) * SUBC)
                    for mo in range(8):
                        ps = ps_qk.tile([128, SUBC], F32, tag="psqk")
                        for kc in range(4):
                            nc.tensor.matmul(
                                ps[:],
                                lhsT=wt[kc][:, mo * 128:(mo + 1) * 128],
                                rhs=xt[kc][:, cs],
                                start=(kc == 0),
                                stop=(kc == 3),
                            )
                        if mo < 4:  # q: add bias on ScalarE while copying out
                            nc.scalar.activation(
                                q_sb[mo][:, cs], ps[:], AF.Identity, bias=bq[mo][:]
                            )
                        else:  # k: add (rel bias + k bias)
                            nc.vector.tensor_tensor(
                                k_sb[mo - 4][:, cs], ps[:], rpw[mo - 4][:],
                                AluOpType.add,
                            )

                # v token-major GEMM with cross-batch 128-token stationary
                # tiles; epilogue writes interleaved [tok, h, d|1] bf16
                vtoks = []   # (tok0, tile)
                for t0 in range(0, NTOK, 128):
                    t1 = min(t0 + 128, NTOK)
                    w = t1 - t0
                    ps = ps_v.tile([128, DM], F32, tag="psv")
                    for kc in range(4):
                        nc.tensor.matmul(
                            ps[0:w, :],
                            lhsT=xt[kc][:, t0:t1],
                            rhs=wt[kc][:, 2 * DM:3 * DM],
                            start=(kc == 0),
                            stop=(kc == 3),
                        )
                    vt = vtpool.tile([128, H, E], BF16, tag="vtok")
                    nc.vector.tensor_tensor(
                        vt[0:w, :, 0:D],
                        ps[0:w, :].rearrange("p (h d) -> p h d", h=H),
                        bvb[0:w, :].rearrange("p (h d) -> p h d", h=H),
                        AluOpType.add,
                    )
                    nc.vector.memset(vt[0:w, :, D:E], 1.0)
                    vtoks.append((t0, vt))

                # repartition: per-batch v_aug tiles [81, H*E] carved out of
                # vtok tiles by SBUF->SBUF DMA (full 1040B rows)
                v_aug = []
                for j in range(NB):
                    va = vpool.tile([N, H * E], BF16, tag=f"vaug{j}")
                    r0 = j * N
                    for t0, vt in vtoks:
                        lo = max(r0, t0)
                        hi = min(r0 + N, t0 + 128)
                        if lo < hi:
                            nc.sync.dma_start(
                                out=va[lo - r0:hi - r0, :],
                                in_=vt[lo - t0:hi - t0, :, :]
                                .rearrange("p h e -> p (h e)"),
                            )
                    v_aug.append(va)
                return {"q": q_sb, "k": k_sb, "v": v_aug, "b0": b0}

            def attention(st):
                q_sb, k_sb, v_aug, b0 = st["q"], st["k"], st["v"], st["b0"]
                ot = None
                for j in range(NB):
                    b = b0 + j
                    js = slice(j * N, (j + 1) * N)
                    if j % 2 == 0:  # one output tile per batch pair
                        ot = opool.tile([D + 1, 2 * 2 * 4 * N], BF16, tag="ot")
                    # scores transposed: S^T = k'.T-contracted over d.
                    # Grouped by head parity: tile `par` holds heads 2*hh+par,
                    # so every matmul into one PSUM tile has the same lhsT
                    # base partition; parities interleaved: consecutive
                    # matmuls use disjoint PE row strips (0-63 vs 64-127) and
                    # different PSUM banks, so the PE can overlap them
                    psS = [
                        ps_s.tile([N, 4 * N], F32, tag="pss", name=f"pss{j}_{p}")
                        for p in range(2)
                    ]
                    for hh in range(4):
                        for par in range(2):
                            po = par * 64
                            nc.tensor.matmul(
                                psS[par][:, hh * N:(hh + 1) * N],
                                lhsT=k_sb[hh][po:po + 64, js],
                                rhs=q_sb[hh][po:po + 64, js],
                                start=True,
                                stop=True,
                                tile_position=(po, 0),
                            )
                    emat = []
                    for par in range(2):
                        e = epool.tile([N, 4 * N], BF16, tag="e", name=f"e{par}")
                        nc.scalar.activation(e[:], psS[par][:], AF.Exp)
                        emat.append(e)
                    # AV with ones-row: rows 0..63 unnormalized out, row 64 denom
                    psA = []
                    for par in range(2):
                        ps = ps_av.tile([D + 1, 4 * N], F32, tag="psav")
                        for hh in range(4):
                            h = 2 * hh + par
                            nc.tensor.matmul(
                                ps[:, hh * N:(hh + 1) * N],
                                lhsT=v_aug[j][:, h * E:(h + 1) * E],
                                rhs=emat[par][:, hh * N:(hh + 1) * N],
                                start=True,
                                stop=True,
                            )
                        psA.append(ps)
                    # ot free layout is (b01, par, hh, n); channel h = 2*hh+par
                    # one copy on DVE, one on ACT to balance engine load
                    joff = (j % 2) * 2 * 4 * N
                    nc.vector.tensor_copy(
                        ot[:, joff:joff + 4 * N], psA[0][:]
                    )
                    nc.scalar.activation(
                        ot[:, joff + 4 * N:joff + 2 * 4 * N], psA[1][:], AF.Identity
                    )
                    if j % 2 == 1:
                        nc.sync.dma_start(out=outd[b // 2], in_=ot[:])

            # software pipeline: attention for chunk c-1 is emitted before
            # GEMM for chunk c so PE never stalls on ACT/DVE epilogues
            for c in range(nchunks + 1):
                if c > 0:
                    attention(state)
                if c < nchunks:
                    state = gemm(c)

    if not nc.is_finalized():
        nc.finalize()
    return nc


_CACHE = {}


def _get_nc(n_b):
    if n_b not in _CACHE:
        _CACHE[n_b] = build_kernel(n_b)
    return _CACHE[n_b]


def _prep_inputs(x, qkv_w, qkv_b, rel_h, rel_w):
    # per-core channel-major x: [NCORES][DM, B_CORE*N] in fp16
    x = np.asarray(x, dtype=np.float32).reshape(B, DM, N)
    x = np.ascontiguousarray(
        x.reshape(NCORES, B_CORE, DM, N).transpose(0, 2, 1, 3)
    ).reshape(NCORES, DM, B_CORE * N).astype(np.float16)
    qkv_w = np.asarray(qkv_w, dtype=np.float32)
    qkv_b = np.asarray(qkv_b, dtype=np.float32)
    wt = np.ascontiguousarray(qkv_w.T).astype(np.float16)                # [512, 1536]
    bq = np.ascontiguousarray(qkv_b[0:DM].reshape(DM, 1))
    rel = (np.asarray(rel_h, np.float32) + np.asarray(rel_w, np.float32))
    rp = np.ascontiguousarray(rel.reshape(DM, N) + qkv_b[DM:2 * DM].reshape(DM, 1))
    bv = np.ascontiguousarray(qkv_b[2 * DM:3 * DM].reshape(1, DM))
    return x, wt, bq, rp, bv


def kernel(x, qkv_w, qkv_b, rel_h, rel_w, _trace=False):
    xs, wt, bq, rp, bv = _prep_inputs(x, qkv_w, qkv_b, rel_h, rel_w)
    nc = _get_nc(B_CORE)
    in_maps = [
        {"x": xs[i], "wt": wt, "bq": bq, "rp": rp, "bv": bv}
        for i in range(NCORES)
    ]
    res = run_bass_kernel_spmd(
        nc, in_maps, core_ids=list(range(NCORES)), trace=_trace
    )
    # decode device layout [pair, d|denom, b01, par, hh, n] -> [B, DM, N];
    # row D is the softmax denominator (normalize here during unshard)
    out = np.stack(
        [np.asarray(r["out"]).astype(np.float32) for r in res.results], axis=0
    )
    out = out.reshape(NCORES, B_CORE // 2, D + 1, 2, 2, 4, N)
    out = out[:, :, 0:D] / out[:, :, D:D + 1]
    out = out.transpose(0, 1, 3, 5, 4, 2, 6)  # core, pair, b01, hh, par, d, n
    out = out.reshape(B, DM, N)
    if _trace:
        kernel.last_results = res
    return np.ascontiguousarray(out.reshape(B, DM, 9, 9))


# revision 20
# speedup vs baseline: 1.5118x; 1.1747x over previous
"""Trainium2 Bass kernel for batched 9x9-token MHSA with decomposed relative
position bias (1x1-conv QKV projection).

Strategy: pure data parallel over batch (B=1024 -> 128 per core x 8 cores).
Per core (all-fp16 GEMM datapath, fp32 PSUM accumulation):
  - QK projection GEMM channel-major (out [o, (b,n)]), fp16, N=324 tiles.
    Relative-position table R = rel_h+rel_w (+ k bias) is folded into K
    during the PSUM->SBUF epilogue, so scores = Q.(K+R) in one matmul.
  - V projection GEMM token-major with cross-batch 128-token stationary
    tiles (full PE width), epilogue writes interleaved [tok, h, d+1] bf16
    with a ones column; per-batch v_aug tiles are then carved out by
    SBUF->SBUF repartition DMAs (contiguous 1040B rows).
  - Scores computed transposed: S^T[m,n] = sum_d k'[d,m] q[d,n] via
    matmul(lhsT=k', rhs=q), fp16 inputs. Softmax runs over partitions (m):
    no max subtraction (logits bounded ~33, exp fits fp32/bf16 range);
    denominator obtained from the ones column of v_aug so the AV matmul
    emits unnormalized output rows 0..63 and the denominator in row 64.
  - exp on ScalarE (fp32 PSUM -> bf16 SBUF), AV matmul in bf16.
  - Final normalize (divide by denominator row) happens on the host during
    unsharding.

Self-contained: hardcodes B=1024, DM=512, H=8, D=64, N=81, 8 cores.
"""

import os
import sys

import numpy as np

for _p in ("/opt/trn_rl_repo", "/root/.axon_site/_ro/trn_rl_repo"):
    if os.path.isdir(_p) and _p not in sys.path:
        sys.path.insert(0, _p)

import concourse.bass as bass
import concourse.tile as tile
from concourse import bacc
from concourse import mybir
from concourse.alu_op_type import AluOpType
from concourse.bass_utils import run_bass_kernel_spmd

F32 = mybir.dt.float32
BF16 = mybir.dt.bfloat16
FP16 = mybir.dt.float16
AF = mybir.ActivationFunctionType

B, DM, H, D, N = 1024, 512, 8, 64, 81
NCORES = 8
B_CORE = B // NCORES   # 128
NB = 16                # batches per chunk
NTOK = NB * N          # 1296 tokens per chunk
NSUB = 3               # QK sub-chunks per chunk
SUBC = NTOK // NSUB    # 432 moving columns per QK matmul (<=512 fp32 PSUM)
E = D + 1              # 65: v columns + ones (denominator) column


def build_kernel(n_b=B_CORE):
    assert n_b % NB == 0
    nchunks = n_b // NB

    nc = bacc.Bacc()
    # x pre-transposed on host to channel-major [DM, n_b*N] fp16 (contiguous
    # 2592B DMA rows per chunk slice; fp16 keeps every matmul at the
    # 1-cycle/row rate and halves HBM traffic vs fp32).
    xd = nc.dram_tensor("x", [DM, n_b * N], FP16, kind="ExternalInput")
    wtd = nc.dram_tensor("wt", [DM, 3 * DM], FP16, kind="ExternalInput")  # W^T
    bqd = nc.dram_tensor("bq", [DM, 1], F32, kind="ExternalInput")        # q bias
    # rel position table pre-tiled on host over one chunk of columns so the
    # k epilogue reads a plain [128, SUBC] slice at any sub-chunk phase
    rpd = nc.dram_tensor("rp", [DM, NTOK], F32, kind="ExternalInput")
    bvd = nc.dram_tensor("bv", [1, DM], F32, kind="ExternalInput")        # v bias row
    # out in device-native layout [pair][d+denom][b par hh n]; row D holds the
    # softmax denominator — the final normalize division happens on the host
    # during unsharding. One fully-contiguous store per batch pair.
    outd = nc.dram_tensor(
        "out", [n_b // 2, D + 1, 2 * 2 * 4 * N], BF16, kind="ExternalOutput"
    )

    with tile.TileContext(nc) as tc:
        with (
            tc.tile_pool(name="const", bufs=1) as cpool,
            tc.tile_pool(name="xin", bufs=3) as xpool,
            tc.tile_pool(name="qk", bufs=2) as qkpool,
            tc.tile_pool(name="vtok", bufs=4) as vtpool,
            tc.tile_pool(name="vaug", bufs=2) as vpool,
            tc.tile_pool(name="emat", bufs=4) as epool,
            tc.tile_pool(name="outs", bufs=3) as opool,
            tc.tile_pool(name="ps_qk", bufs=2, space="PSUM") as ps_qk,
            tc.tile_pool(name="ps_v", bufs=2, space="PSUM") as ps_v,
            tc.tile_pool(name="ps_s", bufs=2, space="PSUM") as ps_s,
            tc.tile_pool(name="ps_av", bufs=2, space="PSUM") as ps_av,
        ):
            # ---- constants (loaded once) ----
            wt = []
            for kc in range(4):
                t = cpool.tile([128, 3 * DM], FP16, tag=f"wt{kc}")
                nc.sync.dma_start(out=t[:], in_=wtd[kc * 128:(kc + 1) * 128, :])
                wt.append(t)
            bq = []
            for mo in range(4):
                t = cpool.tile([128, 1], F32, tag=f"bq{mo}")
                nc.sync.dma_start(out=t[:], in_=bqd[mo * 128:(mo + 1) * 128, :])
                bq.append(t)
            rpw = []
            for mo in range(4):
                row = []
                for s in range(NSUB):
                    t = cpool.tile([128, SUBC], F32, tag=f"rpw{mo}_{s}",
                                   name=f"rpw{mo}_{s}")
                    nc.sync.dma_start(
                        out=t[:],
                        in_=rpd[mo * 128:(mo + 1) * 128,
                                s * SUBC:(s + 1) * SUBC],
                    )
                    row.append(t)
                rpw.append(row)
            bvb = cpool.tile([128, DM], F32, tag="bvb")
            nc.sync.dma_start(out=bvb[:], in_=bvd[0:1, :].to_broadcast([128, DM]))

            state = {}  # carries one chunk's tiles to the next iteration

            def gemm(c):
                b0 = c * NB
                xt = []
                for kc in range(4):
                    t = xpool.tile([128, NTOK], FP16, tag=f"x{kc}")
                    nc.sync.dma_start(
                        out=t[:],
                        in_=xd[kc * 128:(kc + 1) * 128, b0 * N:(b0 + NB) * N],
                    )
                    xt.append(t)

                # q,k channel-major GEMM: out[o, (b,n)] for o in 0..1024,
                # NSUB sub-chunks of SUBC moving columns each
                q_sb = [
                    qkpool.tile([128, NTOK], FP16, tag=f"q{mo}", name=f"q{mo}")
                    for mo in range(4)
                ]
                k_sb = [
                    qkpool.tile([128, NTOK], FP16, tag=f"k{mo}", name=f"k{mo}")
                    for mo in range(4)
                ]
                for s in range(NSUB):
                    cs = slice(s * SUBC, (s + 1) * SUBC)
                    for mo in range(8):
                        ps = ps_qk.tile([128, SUBC], F32, tag="psqk")
                        for kc in range(4):
                            nc.tensor.matmul(
                                ps[:],
                                lhsT=wt[kc][:, mo * 128:(mo + 1) * 128],
                                rhs=xt[kc][:, cs],
                                start=(kc == 0),
                                stop=(kc == 3),
                            )
                        if mo < 4:  # q: add bias on ScalarE while copying out
                            nc.scalar.activation(
                                q_sb[mo][:, cs], ps[:], AF.Identity, bias=bq[mo][:]
                            )
                        else:  # k: add (rel bias + k bias)
                            nc.vector.tensor_tensor(
                                k_sb[mo - 4][:, cs], ps[:], rpw[mo - 4][s],
                                AluOpType.add,
                            )

                # v token-major GEMM with cross-batch 128-token stationary
                # tiles; epilogue writes interleaved [tok, h, d|1] bf16
                vtoks = []   # (tok0, tile)
                for t0 in range(0, NTOK, 128):
                    t1 = min(t0 + 128, NTOK)
                    w = t1 - t0
                    ps = ps_v.tile([128, DM], F32, tag="psv")
                    for kc in range(4):
                        nc.tensor.matmul(
                            ps[0:w, :],
                            lhsT=xt[kc][:, t0:t1],
                            rhs=wt[kc][:, 2 * DM:3 * DM],
                            start=(kc == 0),
                            stop=(kc == 3),
                        )
                    vt = vtpool.tile([128, H, E], BF16, tag="vtok")
                    nc.vector.tensor_tensor(
                        vt[0:w, :, 0:D],
                        ps[0:w, :].rearrange("p (h d) -> p h d", h=H),
                        bvb[0:w, :].rearrange("p (h d) -> p h d", h=H),
                        AluOpType.add,
                    )
                    nc.vector.memset(vt[0:w, :, D:E], 1.0)
                    vtoks.append((t0, vt))

                # repartition: per-batch v_aug tiles [81, H*E] carved out of
                # vtok tiles by SBUF->SBUF DMA (full 1040B rows)
                v_aug = []
                for j in range(NB):
                    va = vpool.tile([N, H * E], BF16, tag=f"vaug{j}")
                    r0 = j * N
                    for t0, vt in vtoks:
                        lo = max(r0, t0)
                        hi = min(r0 + N, t0 + 128)
                        if lo < hi:
                            nc.sync.dma_start(
                                out=va[lo - r0:hi - r0, :],
                                in_=vt[lo - t0:hi - t0, :, :]
                                .rearrange("p h e -> p (h e)"),
                            )
                    v_aug.append(va)
                return {"q": q_sb, "k": k_sb, "v": v_aug, "b0": b0}

            def attention(st):
                q_sb, k_sb, v_aug, b0 = st["q"], st["k"], st["v"], st["b0"]
                ot = None
                for j in range(NB):
                    b = b0 + j
                    js = slice(j * N, (j + 1) * N)
                    if j % 2 == 0:  # one output tile per batch pair
                        ot = opool.tile([D + 1, 2 * 2 * 4 * N], BF16, tag="ot")
                    # scores transposed: S^T = k'.T-contracted over d.
                    # Grouped by head parity: tile `par` holds heads 2*hh+par,
                    # so every matmul into one PSUM tile has the same lhsT
                    # base partition; parities interleaved: consecutive
                    # matmuls use disjoint PE row strips (0-63 vs 64-127) and
                    # different PSUM banks, so the PE can overlap them
                    psS = [
                        ps_s.tile([N, 4 * N], F32, tag="pss", name=f"pss{j}_{p}")
                        for p in range(2)
                    ]
                    for hh in range(4):
                        for par in range(2):
                            po = par * 64
                            nc.tensor.matmul(
                                psS[par][:, hh * N:(hh + 1) * N],
                                lhsT=k_sb[hh][po:po + 64, js],
                                rhs=q_sb[hh][po:po + 64, js],
                                start=True,
                                stop=True,
                                tile_position=(po, 0),
                            )
                    emat = []
                    for par in range(2):
                        e = epool.tile([N, 4 * N], BF16, tag="e", name=f"e{par}")
                        nc.scalar.activation(e[:], psS[par][:], AF.Exp)
                        emat.append(e)
                    # AV with ones-row: rows 0..63 unnormalized out, row 64 denom
                    psA = []
                    for par in range(2):
                        ps = ps_av.tile([D + 1, 4 * N], F32, tag="psav")
                        for hh in range(4):
                            h = 2 * hh + par
                            nc.tensor.matmul(
                                ps[:, hh * N:(hh + 1) * N],
                                lhsT=v_aug[j][:, h * E:(h + 1) * E],
                                rhs=emat[par][:, hh * N:(hh + 1) * N],
                                start=True,
                                stop=True,
                            )
                        psA.append(ps)
                    # ot free layout is (b01, par, hh, n); channel h = 2*hh+par
                    # one copy on DVE, one on ACT to balance engine load
                    joff = (j % 2) * 2 * 4 * N
                    nc.vector.tensor_copy(
                        ot[:, joff:joff + 4 * N], psA[0][:]
                    )
                    nc.scalar.activation(
                        ot[:, joff + 4 * N:joff + 2 * 4 * N], psA[1][:], AF.Identity
                    )
                    if j % 2 == 1:
                        nc.sync.dma_start(out=outd[b // 2], in_=ot[:])

            # software pipeline: attention for chunk c-1 is emitted before
            # GEMM for chunk c so PE never stalls on ACT/DVE epilogues
            for c in range(nchunks + 1):
                if c > 0:
                    attention(state)
                if c < nchunks:
                    state = gemm(c)

    if not nc.is_finalized():
        nc.finalize()
    return nc


_CACHE = {}


def _get_nc(n_b):
    if n_b not in _CACHE:
        _CACHE[n_b] = build_kernel(n_b)
    return _CACHE[n_b]


def _prep_inputs(x, qkv_w, qkv_b, rel_h, rel_w):
    # per-core channel-major x: [NCORES][DM, B_CORE*N] in fp16
    x = np.asarray(x, dtype=np.float32).reshape(B, DM, N)
    x = np.ascontiguousarray(
        x.reshape(NCORES, B_CORE, DM, N).transpose(0, 2, 1, 3)
    ).reshape(NCORES, DM, B_CORE * N).astype(np.float16)
    qkv_w = np.asarray(qkv_w, dtype=np.float32)
    qkv_b = np.asarray(qkv_b, dtype=np.float32)
    wt = np.ascontiguousarray(qkv_w.T).astype(np.float16)                # [512, 1536]
    bq = np.ascontiguousarray(qkv_b[0:DM].reshape(DM, 1))
    rel = (np.asarray(rel_h, np.float32) + np.asarray(rel_w, np.float32))
    rp = rel.reshape(DM, N) + qkv_b[DM:2 * DM].reshape(DM, 1)
    rp = np.ascontiguousarray(np.tile(rp, (1, NB)))   # [DM, NB*N]
    bv = np.ascontiguousarray(qkv_b[2 * DM:3 * DM].reshape(1, DM))
    return x, wt, bq, rp, bv


def kernel(x, qkv_w, qkv_b, rel_h, rel_w, _trace=False):
    xs, wt, bq, rp, bv = _prep_inputs(x, qkv_w, qkv_b, rel_h, rel_w)
    nc = _get_nc(B_CORE)
    in_maps = [
        {"x": xs[i], "wt": wt, "bq": bq, "rp": rp, "bv": bv}
        for i in range(NCORES)
    ]
    res = run_bass_kernel_spmd(
        nc, in_maps, core_ids=list(range(NCORES)), trace=_trace
    )
    # decode device layout [pair, d|denom, b01, par, hh, n] -> [B, DM, N];
    # row D is the softmax denominator (normalize here during unshard)
    out = np.stack(
        [np.asarray(r["out"]).astype(np.float32) for r in res.results], axis=0
    )
    out = out.reshape(NCORES, B_CORE // 2, D + 1, 2, 2, 4, N)
    out = out[:, :, 0:D] / out[:, :, D:D + 1]
    out = out.transpose(0, 1, 3, 5, 4, 2, 6)  # core, pair, b01, hh, par, d, n
    out = out.reshape(B, DM, N)
    if _trace:
        kernel.last_results = res
    return np.ascontiguousarray(out.reshape(B, DM, 9, 9))


# revision 26
# speedup vs baseline: 1.5291x; 1.0115x over previous
"""Trainium2 Bass kernel for batched 9x9-token MHSA with decomposed relative
position bias (1x1-conv QKV projection).

Strategy: pure data parallel over batch (B=1024 -> 128 per core x 8 cores).
Per core (fp16 GEMM datapath, fp32 PSUM accumulation, bf16 attention/output):
  - QK projection GEMM channel-major (out [o, (b,n)]), fp16, chunks of 16
    batches processed in 4 sub-chunks of 324 moving columns. Relative
    position table R = rel_h+rel_w (+ k bias) is folded into K during the
    PSUM->SBUF epilogue, so scores = Q.(K+R) in one matmul. fp16 keeps every
    matmul at the 1-cycle/row PE rate (fp32 is 4 cycles/row; fp32r is
    4 cycles/row below 256 moving columns) with ~4x the mantissa of bf16.
  - V projection GEMM token-major with cross-batch 128-token stationary
    tiles (full PE width), epilogue writes interleaved [tok, h, d|1] bf16
    with a ones column; per-batch v_aug tiles are then carved out by
    SBUF->SBUF repartition DMAs (contiguous 1040B rows).
  - Scores computed transposed: S^T[m,n] = sum_d k'[d,m] q[d,n] via
    matmul(lhsT=k', rhs=q), fp16 inputs, two head-parity PSUM tiles on
    disjoint PE row strips (tile_position) so pairs of matmuls overlap.
    Softmax runs over partitions (m): no max subtraction (logits bounded
    ~33, exp fits fp32/bf16 range); denominator comes from the ones column
    of v_aug so the AV matmul emits unnormalized output rows 0..63 and the
    denominator in row 64.
  - exp on ScalarE (fp32 PSUM -> bf16 SBUF), AV matmul in bf16, output
    staged bf16 and stored bf16 (halves store traffic + copy cost).
  - Final normalize (divide by denominator row) happens on the host during
    unsharding; host also pre-transposes x to channel-major fp16 and folds
    biases.

Self-contained: hardcodes B=1024, DM=512, H=8, D=64, N=81, 8 cores.
"""

import os
import sys

import numpy as np

for _p in ("/opt/trn_rl_repo", "/root/.axon_site/_ro/trn_rl_repo"):
    if os.path.isdir(_p) and _p not in sys.path:
        sys.path.insert(0, _p)

import concourse.bass as bass
import concourse.tile as tile
from concourse import bacc
from concourse import mybir
from concourse.alu_op_type import AluOpType
from concourse.bass_utils import run_bass_kernel_spmd

F32 = mybir.dt.float32
BF16 = mybir.dt.bfloat16
FP16 = mybir.dt.float16
AF = mybir.ActivationFunctionType

B, DM, H, D, N = 1024, 512, 8, 64, 81
NCORES = 8
B_CORE = B // NCORES   # 128
NB = 16                # batches per chunk
NTOK = NB * N          # 1296 tokens per chunk
NSUB = 4               # QK sub-chunks per chunk
SUBC = NTOK // NSUB    # 324 moving columns per QK matmul
E = D + 1              # 65: v columns + ones (denominator) column


def build_kernel(n_b=B_CORE):
    assert n_b % NB == 0
    nchunks = n_b // NB

    nc = bacc.Bacc()
    # x pre-transposed on host to channel-major [DM, n_b*N] fp16 (contiguous
    # 2592B DMA rows per chunk slice; fp16 keeps every matmul at the
    # 1-cycle/row rate and halves HBM traffic vs fp32).
    xd = nc.dram_tensor("x", [DM, n_b * N], FP16, kind="ExternalInput")
    wtd = nc.dram_tensor("wt", [DM, 3 * DM], FP16, kind="ExternalInput")  # W^T
    bqd = nc.dram_tensor("bq", [DM, 1], F32, kind="ExternalInput")        # q bias
    rpd = nc.dram_tensor("rp", [DM, N], F32, kind="ExternalInput")        # rel_h+rel_w+bk
    bvd = nc.dram_tensor("bv", [1, DM], F32, kind="ExternalInput")        # v bias row
    # out in device-native layout [pair][d+denom][b par hh n]; row D holds the
    # softmax denominator — the final normalize division happens on the host
    # during unsharding. One fully-contiguous store per batch pair.
    outd = nc.dram_tensor(
        "out", [n_b // 2, D + 1, 2 * 2 * 4 * N], BF16, kind="ExternalOutput"
    )

    with tile.TileContext(nc) as tc:
        with (
            tc.tile_pool(name="const", bufs=1) as cpool,
            tc.tile_pool(name="xin", bufs=3) as xpool,
            tc.tile_pool(name="qk", bufs=2) as qkpool,
            tc.tile_pool(name="vtok", bufs=4) as vtpool,
            tc.tile_pool(name="vaug", bufs=2) as vpool,
            tc.tile_pool(name="emat", bufs=4) as epool,
            tc.tile_pool(name="outs", bufs=3) as opool,
            tc.tile_pool(name="ps_qk", bufs=2, space="PSUM") as ps_qk,
            tc.tile_pool(name="ps_v", bufs=2, space="PSUM") as ps_v,
            tc.tile_pool(name="ps_s", bufs=2, space="PSUM") as ps_s,
            tc.tile_pool(name="ps_av", bufs=2, space="PSUM") as ps_av,
        ):
            # ---- constants (loaded once) ----
            wt = []
            for kc in range(4):
                t = cpool.tile([128, 3 * DM], FP16, tag=f"wt{kc}")
                nc.sync.dma_start(out=t[:], in_=wtd[kc * 128:(kc + 1) * 128, :])
                wt.append(t)
            bq = []
            for mo in range(4):
                t = cpool.tile([128, 1], F32, tag=f"bq{mo}")
                nc.sync.dma_start(out=t[:], in_=bqd[mo * 128:(mo + 1) * 128, :])
                bq.append(t)
            # rp widened to one QK sub-chunk (4 batches) so the k epilogue is
            # a plain 2D tensor_tensor with no broadcast AP
            rpw = []
            for mo in range(4):
                t = cpool.tile([128, SUBC], F32, tag=f"rpw{mo}")
                nc.sync.dma_start(
                    out=t[:].rearrange("p (b n) -> p b n", b=NSUB),
                    in_=rpd[mo * 128:(mo + 1) * 128, :].unsqueeze(1)
                    .to_broadcast([128, NSUB, N]),
                )
                rpw.append(t)
            bvb = cpool.tile([128, DM], F32, tag="bvb")
            nc.sync.dma_start(out=bvb[:], in_=bvd[0:1, :].to_broadcast([128, DM]))

            state = {}  # carries one chunk's tiles to the next iteration

            def gemm(c):
                b0 = c * NB
                xt = []
                for kc in range(4):
                    t = xpool.tile([128, NTOK], FP16, tag=f"x{kc}")
                    nc.sync.dma_start(
                        out=t[:],
                        in_=xd[kc * 128:(kc + 1) * 128, b0 * N:(b0 + NB) * N],
                    )
                    xt.append(t)

                # q,k channel-major GEMM: out[o, (b,n)] for o in 0..1024,
                # NSUB sub-chunks of SUBC moving columns each
                q_sb = [
                    qkpool.tile([128, NTOK], FP16, tag=f"q{mo}", name=f"q{mo}")
                    for mo in range(4)
                ]
                k_sb = [
                    qkpool.tile([128, NTOK], FP16, tag=f"k{mo}", name=f"k{mo}")
                    for mo in range(4)
                ]
                for s in range(NSUB):
                    cs = slice(s * SUBC, (s + 1) * SUBC)
                    for mo in range(8):
                        ps = ps_qk.tile([128, SUBC], F32, tag="psqk")
                        for kc in range(4):
                            nc.tensor.matmul(
                                ps[:],
                                lhsT=wt[kc][:, mo * 128:(mo + 1) * 128],
                                rhs=xt[kc][:, cs],
                                start=(kc == 0),
                                stop=(kc == 3),
                            )
                        if mo < 4:  # q: add bias on ScalarE while copying out
                            nc.scalar.activation(
                                q_sb[mo][:, cs], ps[:], AF.Identity, bias=bq[mo][:]
                            )
                        else:  # k: add (rel bias + k bias)
                            nc.vector.tensor_tensor(
                                k_sb[mo - 4][:, cs], ps[:], rpw[mo - 4][:],
                                AluOpType.add,
                            )

                # v token-major GEMM with cross-batch 128-token stationary
                # tiles; epilogue writes interleaved [tok, h, d|1] bf16
                vtoks = []   # (tok0, tile)
                for t0 in range(0, NTOK, 128):
                    t1 = min(t0 + 128, NTOK)
                    w = t1 - t0
                    ps = ps_v.tile([128, DM], F32, tag="psv")
                    for kc in range(4):
                        nc.tensor.matmul(
                            ps[0:w, :],
                            lhsT=xt[kc][:, t0:t1],
                            rhs=wt[kc][:, 2 * DM:3 * DM],
                            start=(kc == 0),
                            stop=(kc == 3),
                        )
                    vt = vtpool.tile([128, H, E], BF16, tag="vtok")
                    nc.vector.tensor_tensor(
                        vt[0:w, :, 0:D],
                        ps[0:w, :].rearrange("p (h d) -> p h d", h=H),
                        bvb[0:w, :].rearrange("p (h d) -> p h d", h=H),
                        AluOpType.add,
                    )
                    nc.vector.memset(vt[0:w, :, D:E], 1.0)
                    vtoks.append((t0, vt))

                # repartition: per-batch v_aug tiles [81, H*E] carved out of
                # vtok tiles by SBUF->SBUF DMA (full 1040B rows)
                v_aug = []
                for j in range(NB):
                    va = vpool.tile([N, H * E], BF16, tag=f"vaug{j}")
                    r0 = j * N
                    for t0, vt in vtoks:
                        lo = max(r0, t0)
                        hi = min(r0 + N, t0 + 128)
                        if lo < hi:
                            nc.sync.dma_start(
                                out=va[lo - r0:hi - r0, :],
                                in_=vt[lo - t0:hi - t0, :, :]
                                .rearrange("p h e -> p (h e)"),
                            )
                    v_aug.append(va)
                return {"q": q_sb, "k": k_sb, "v": v_aug, "b0": b0}

            def attention(st):
                q_sb, k_sb, v_aug, b0 = st["q"], st["k"], st["v"], st["b0"]
                ot = None
                for j in range(NB):
                    b = b0 + j
                    js = slice(j * N, (j + 1) * N)
                    if j % 2 == 0:  # one output tile per batch pair
                        ot = opool.tile([D + 1, 2 * 2 * 4 * N], BF16, tag="ot")
                    # scores transposed: S^T = k'.T-contracted over d.
                    # Grouped by head parity: tile `par` holds heads 2*hh+par,
                    # so every matmul into one PSUM tile has the same lhsT
                    # base partition; parities interleaved: consecutive
                    # matmuls use disjoint PE row strips (0-63 vs 64-127) and
                    # different PSUM banks, so the PE can overlap them
                    psS = [
                        ps_s.tile([N, 4 * N], F32, tag="pss", name=f"pss{j}_{p}")
                        for p in range(2)
                    ]
                    for hh in range(4):
                        for par in range(2):
                            po = par * 64
                            nc.tensor.matmul(
                                psS[par][:, hh * N:(hh + 1) * N],
                                lhsT=k_sb[hh][po:po + 64, js],
                                rhs=q_sb[hh][po:po + 64, js],
                                start=True,
                                stop=True,
                                tile_position=(po, 0),
                            )
                    emat = []
                    for par in range(2):
                        e = epool.tile([N, 4 * N], BF16, tag="e", name=f"e{par}")
                        nc.scalar.activation(e[:], psS[par][:], AF.Exp)
                        emat.append(e)
                    # AV with ones-row: rows 0..63 unnormalized out, row 64 denom
                    psA = []
                    for par in range(2):
                        ps = ps_av.tile([D + 1, 4 * N], F32, tag="psav")
                        for hh in range(4):
                            h = 2 * hh + par
                            nc.tensor.matmul(
                                ps[:, hh * N:(hh + 1) * N],
                                lhsT=v_aug[j][:, h * E:(h + 1) * E],
                                rhs=emat[par][:, hh * N:(hh + 1) * N],
                                start=True,
                                stop=True,
                            )
                        psA.append(ps)
                    # ot free layout is (b01, par, hh, n); channel h = 2*hh+par
                    # one copy on DVE, one on ACT to balance engine load
                    joff = (j % 2) * 2 * 4 * N
                    nc.vector.tensor_copy(
                        ot[:, joff:joff + 4 * N], psA[0][:]
                    )
                    nc.scalar.activation(
                        ot[:, joff + 4 * N:joff + 2 * 4 * N], psA[1][:], AF.Identity
                    )
                    if j % 2 == 1:
                        nc.sync.dma_start(out=outd[b // 2], in_=ot[:])

            # software pipeline: attention for chunk c-1 is emitted before
            # GEMM for chunk c so PE never stalls on ACT/DVE epilogues
            for c in range(nchunks + 1):
                if c > 0:
                    attention(state)
                if c < nchunks:
                    state = gemm(c)

    if not nc.is_finalized():
        nc.finalize()
    return nc


_CACHE = {}


def _get_nc(n_b):
    if n_b not in _CACHE:
        _CACHE[n_b] = build_kernel(n_b)
    return _CACHE[n_b]


def _prep_inputs(x, qkv_w, qkv_b, rel_h, rel_w):
    # per-core channel-major x: [NCORES][DM, B_CORE*N] in fp16
    x = np.asarray(x, dtype=np.float32).reshape(B, DM, N)
    x = np.ascontiguousarray(
        x.reshape(NCORES, B_CORE, DM, N).transpose(0, 2, 1, 3)
    ).reshape(NCORES, DM, B_CORE * N).astype(np.float16)
    qkv_w = np.asarray(qkv_w, dtype=np.float32)
    qkv_b = np.asarray(qkv_b, dtype=np.float32)
    wt = np.ascontiguousarray(qkv_w.T).astype(np.float16)                # [512, 1536]
    bq = np.ascontiguousarray(qkv_b[0:DM].reshape(DM, 1))
    rel = (np.asarray(rel_h, np.float32) + np.asarray(rel_w, np.float32))
    rp = np.ascontiguousarray(rel.reshape(DM, N) + qkv_b[DM:2 * DM].reshape(DM, 1))
    bv = np.ascontiguousarray(qkv_b[2 * DM:3 * DM].reshape(1, DM))
    return x, wt, bq, rp, bv


def kernel(x, qkv_w, qkv_b, rel_h, rel_w, _trace=False):
    xs, wt, bq, rp, bv = _prep_inputs(x, qkv_w, qkv_b, rel_h, rel_w)
    nc = _get_nc(B_CORE)
    in_maps = [
        {"x": xs[i], "wt": wt, "bq": bq, "rp": rp, "bv": bv}
        for i in range(NCORES)
    ]
    res = run_bass_kernel_spmd(
        nc, in_maps, core_ids=list(range(NCORES)), trace=_trace
    )
    # decode device layout [pair, d|denom, b01, par, hh, n] -> [B, DM, N];
    # row D is the softmax denominator (normalize here during unshard)
    out = np.stack(
        [np.asarray(r["out"]).astype(np.float32) for r in res.results], axis=0
    )
    out = out.reshape(NCORES, B_CORE // 2, D + 1, 2, 2, 4, N)
    out = out[:, :, 0:D] / out[:, :, D:D + 1]
    out = out.transpose(0, 1, 3, 5, 4, 2, 6)  # core, pair, b01, hh, par, d, n
    out = out.reshape(B, DM, N)
    if _trace:
        kernel.last_results = res
    return np.ascontiguousarray(out.reshape(B, DM, 9, 9))
